# revision 1
# baseline (speedup 1.0000x reference)
# ISTFT kernel for Trainium2 (8 NeuronCores, data-parallel over batch).
#
# Math: out[b, s] for s = 256*c + r (chunk c, offset r) is
#   out[c, r] = sum_{j=0..3} sum_C spec[C, c-j] * invbasis[C, 256*j + r]
# i.e. the overlap-add is folded into 4 shifted matmuls accumulating in PSUM.
# invbasis rows 513 (imag DC) and 1025 (imag Nyquist) are exactly zero
# (pinv of a basis with zero rows), so the contraction packs to exactly
# 1024 = 8 chunks of 128 partitions:
#   packed rows 0..512   = real part rows 0..512   (mag*cos(angle))
#   packed rows 513..1023 = imag part freqs 1..511 (mag*sin(angle))
# Output keeps chunks 4..2047 (the reference trims NFFT=1024 samples per side).
#
# v3: software-pipelined — elementwise for column-slice s overlaps the matmul
# groups of already-completed slices. Range reduction on DVE (tensor_scalar 2x
# mode), sin/cos on ACT, multiplies in bf16 (DVE 2x). mag is cast to bf16
# during the DMA itself (SWDGE, issued ahead of everything else); invbasis
# loads f32 over HWDGE and is cast to bf16 on the (initially idle) DVE.
import numpy as np

import concourse.bacc as bacc
import concourse.mybir as mybir
import concourse.tile as tile
from concourse.bass_utils import run_bass_kernel_spmd

F32 = mybir.dt.float32
BF16 = mybir.dt.bfloat16
ALU = mybir.AluOpType
ACTF = mybir.ActivationFunctionType

TWO_PI = 6.283185307179586
INV_2PI = 1.0 / TWO_PI
MAGIC = 12582912.0  # 1.5 * 2**23, forces round-to-nearest-int in fp32
PI = 3.141592653589793
HALF_PI = PI / 2
SIN_SCALE = 0.999999  # keeps rounding fuzz at +/-pi inside the Sin table domain

B_LOCAL = 2      # batches per core (16 total / 8 cores)
T = 2048         # STFT frames
SLICES = [(0, 768), (768, 768), (1536, 512)]  # elementwise column slices
N_CORES = 8


def build_nc(repeat=1):
    nc = bacc.Bacc(target_bir_lowering=False)
    mag = nc.declare_dram_parameter("mag", [B_LOCAL, 513, T], BF16, isOutput=False)
    ang = nc.declare_dram_parameter("angle", [B_LOCAL, 513, T], F32, isOutput=False)
    invb = nc.declare_dram_parameter("invbasis", [1024, 1024], BF16, isOutput=False)
    out = nc.declare_dram_parameter("out", [B_LOCAL, 523008], F32, isOutput=True)

    with tile.TileContext(nc) as tc:
        with (
            tc.tile_pool(name="const", bufs=1) as constp,
            tc.tile_pool(name="stage", bufs=4) as stagep,
            tc.tile_pool(name="spec", bufs=2) as specp,
            tc.tile_pool(name="work", bufs=3) as workp,
            tc.tile_pool(name="osb", bufs=4) as osbp,
            tc.tile_pool(name="psum", bufs=8, space="PSUM") as psump,
        ):
            ib = [
                constp.tile([128, 1024], BF16, tag=f"ib{q}", name=f"ib{q}")
                for q in range(8)
            ]
            def load_invb():
                # invbasis arrives pre-packed/bf16 from the host: 8 plain
                # loads on the (otherwise idle) scalar HWDGE ring.
                for q in range(8):
                    nc.scalar.dma_start(
                        out=ib[q][:, :], in_=invb[128 * q : 128 * (q + 1), :]
                    )

            def emit_group(b, spec, ct):
                """32 accumulating matmuls -> one [128,256] output tile -> DRAM."""
                ps = psump.tile([128, 256], F32, tag="ps", name="ps")
                c0 = 128 * ct
                mmi = 0
                for q in range(8):
                    for j in range(4):
                        nc.tensor.matmul(
                            out=ps[:, :],
                            lhsT=spec[q][:, c0 - j + 4 : c0 - j + 132],
                            rhs=ib[q][:, 256 * j : 256 * (j + 1)],
                            start=(mmi == 0),
                            stop=(mmi == 31),
                        )
                        mmi += 1
                ob = osbp.tile([128, 256], F32, tag="ob", name="ob")
                nc.scalar.activation(ob[:, :], ps[:, :], ACTF.Copy)
                eng = nc.sync if ct % 2 == 0 else nc.scalar
                if ct == 0:
                    eng.dma_start(out=out[b, 0:31744], in_=ob[4:128, :])
                elif ct == 15:
                    lo = 256 * (128 * 15 - 4)
                    eng.dma_start(out=out[b, lo : lo + 32512], in_=ob[0:127, :])
                else:
                    lo = 256 * (128 * ct - 4)
                    eng.dma_start(out=out[b, lo : lo + 32768], in_=ob[:, :])

            load_invb()
            for b in [b for _ in range(repeat) for b in range(B_LOCAL)]:
                # --- spec tiles: bf16 [128, 4 + T], 4 leading zero columns ---
                spec = []
                for q in range(8):
                    st = specp.tile([128, 4 + T], BF16, tag=f"spec{q}", name=f"spec{q}")
                    nc.vector.memset(st[:, 0:4], 0.0)
                    spec.append(st)

                emitted = 0
                for si, (s, W) in enumerate(SLICES):
                    cs = slice(4 + s, 4 + s + W)
                    # --- elementwise: rows 0..511 in 4 chunks of 128 ---
                    for q in range(4):
                        rows = slice(128 * q, 128 * (q + 1))
                        mt = stagep.tile([128, W], BF16, tag="mt", name="mt")
                        at = stagep.tile([128, W], F32, tag="at", name="at")
                        nc.sync.dma_start(out=mt[:, :], in_=mag[b, rows, s : s + W])
                        nc.sync.dma_start(out=at[:, :], in_=ang[b, rows, s : s + W])

                        tb = workp.tile([128, W], F32, tag="tb", name="tb")
                        nc.vector.tensor_scalar(
                            out=tb, in0=at, scalar1=INV_2PI, scalar2=MAGIC,
                            op0=ALU.mult, op1=ALU.add,
                        )
                        kk = workp.tile([128, W], F32, tag="kk", name="kk")
                        nc.vector.tensor_scalar_sub(kk, tb, MAGIC)
                        red = workp.tile([128, W], F32, tag="red", name="red")
                        # red = angle - k*2pi  in [-pi, pi] (+tiny rounding fuzz)
                        nc.vector.scalar_tensor_tensor(
                            out=red, in0=kk, scalar=-TWO_PI, in1=at,
                            op0=ALU.mult, op1=ALU.add,
                        )
                        sv = workp.tile([128, W], BF16, tag="sv", name="sv")
                        nc.scalar.activation(sv, red, ACTF.Sin, scale=SIN_SCALE)
                        # imag rows: freq f -> spec[4+q] partition f%128. For q=0,
                        # partition 0 (freq 0) is garbage here; the row-512 pass
                        # below overwrites it with the real Nyquist row 512.
                        nc.vector.tensor_mul(spec[4 + q][:, cs], mt, sv)
                        redc = workp.tile([128, W], F32, tag="redc", name="redc")
                        nc.vector.add_range_wrap(
                            out=redc, in_=red, shift=HALF_PI, bound=PI, period=TWO_PI
                        )
                        cv = workp.tile([128, W], BF16, tag="cv", name="cv")
                        nc.scalar.activation(cv, redc, ACTF.Sin, scale=SIN_SCALE)
                        nc.vector.tensor_mul(spec[q][:, cs], mt, cv)


                    # --- row 512 (real only) -> spec[4] partition 0 ---
                    m5 = stagep.tile([1, W], BF16, tag="mt", name="m5")
                    a5 = stagep.tile([1, W], F32, tag="at", name="a5")
                    nc.sync.dma_start(out=m5[:, :], in_=mag[b, 512:513, s : s + W])
                    nc.sync.dma_start(out=a5[:, :], in_=ang[b, 512:513, s : s + W])
                    tb5 = workp.tile([1, W], F32, tag="tb", name="tb5")
                    nc.vector.tensor_scalar(
                        out=tb5, in0=a5, scalar1=INV_2PI, scalar2=MAGIC,
                        op0=ALU.mult, op1=ALU.add,
                    )
                    kk5 = workp.tile([1, W], F32, tag="kk", name="kk5")
                    nc.vector.tensor_scalar_sub(kk5, tb5, MAGIC)
                    red5 = workp.tile([1, W], F32, tag="red", name="red5")
                    nc.vector.scalar_tensor_tensor(
                        out=red5, in0=kk5, scalar=-TWO_PI, in1=a5,
                        op0=ALU.mult, op1=ALU.add,
                    )
                    redc5 = workp.tile([1, W], F32, tag="redc", name="redc5")
                    nc.vector.add_range_wrap(
                        out=redc5, in_=red5, shift=HALF_PI, bound=PI, period=TWO_PI
                    )
                    cv5 = workp.tile([1, W], BF16, tag="cv", name="cv5")
                    nc.scalar.activation(cv5, redc5, ACTF.Sin, scale=SIN_SCALE)
                    nc.vector.tensor_mul(spec[4][0:1, cs], m5, cv5)

                    # --- matmul groups whose spec columns are now complete ---
                    # group ct reads spec cols (128*ct+1 .. 128*ct+135); those are
                    # ready once cols < 4+s+W are written.
                    hi = min(16, (s + W - 132) // 128 + 1)
                    for ct in range(emitted, hi):
                        emit_group(b, spec, ct)
                    emitted = hi

                for ct in range(emitted, 16):
                    emit_group(b, spec, ct)
    nc.compile()
    return nc


_CACHE = {}


def _get_nc():
    if "nc" not in _CACHE:
        _CACHE["nc"] = build_nc()
    return _CACHE["nc"]


def make_in_maps(mag, angle, invbasis):
    """Host-side input marshalling: shard over cores, bf16-convert mag, and
    pre-pack invbasis to the 1024-row bf16 layout (drops the two all-zero
    rows 513/1025; packed row 512+f <-> imag freq f)."""
    import ml_dtypes

    bf16 = ml_dtypes.bfloat16
    mag = np.asarray(mag, dtype=np.float32).astype(bf16)
    angle = np.ascontiguousarray(np.asarray(angle, dtype=np.float32))
    invbasis = np.asarray(invbasis, dtype=np.float32)
    invb_packed = np.ascontiguousarray(
        np.concatenate([invbasis[:513], invbasis[514:1025]], axis=0).astype(bf16)
    )
    return [
        {
            "mag": np.ascontiguousarray(mag[B_LOCAL * i : B_LOCAL * (i + 1)]),
            "angle": angle[B_LOCAL * i : B_LOCAL * (i + 1)],
            "invbasis": invb_packed,
        }
        for i in range(N_CORES)
    ]


def kernel(mag, angle, invbasis, _trace=False, **_ignored):
    nc = _get_nc()
    in_maps = make_in_maps(mag, angle, invbasis)
    res = run_bass_kernel_spmd(nc, in_maps, list(range(N_CORES)), trace=_trace)
    outs = [res.results[i]["out"] for i in range(N_CORES)]
    full = np.concatenate(outs, axis=0).reshape(16, 1, 523008)
    if _trace:
        return full, res
    return full



# revision 5
# speedup vs baseline: 1.0355x; 1.0355x over previous
# ISTFT kernel for Trainium2 (8 NeuronCores, data-parallel over batch).
#
# Math: out[b, s] for s = 256*c + r (chunk c, offset r) is
#   out[c, r] = sum_{j=0..3} sum_C spec[C, c-j] * invbasis[C, 256*j + r]
# i.e. the overlap-add is folded into 4 shifted matmuls accumulating in PSUM.
# invbasis rows 513 (imag DC) and 1025 (imag Nyquist) are exactly zero, so the
# contraction packs to exactly 1024 = 8 chunks of 128 partitions:
#   packed rows 0..511    = real rows 0..511   (mag*cos(angle))
#   packed row 512        = real row 512 (Nyquist)
#   packed rows 513..1023 = imag freqs 1..511  (mag*sin(angle))
#
# v4: all-fp16 datapath. fp16 matmuls run at the same rate as bf16 (1 col/cyc)
# with 3 more mantissa bits; fp16 also unlocks the DVE 2x perf mode for the
# whole elementwise chain:
#   tb  = a*(1/2pi) + 1536        (fp16 magic rounding, ulp(1536)=1)
#   kk  = tb - 1536               (= round(a/2pi), exact small int)
#   red = a - 2pi*kk              (scalar_tensor_tensor, f32 internal)
#   sin = Sin(red*s)              (ACT, s=0.999999 keeps +-pi in table domain)
#   cos = Sin(pi/2*s - s*|red|)   (|red| via fp16 sign-bit mask; cos(x)=sin(pi/2-|x|))
#   spec = (1*mag) * {sin,cos}    (scalar_tensor_tensor, fp16 2x)
# Row 512 (real-only) is computed once per batch up-front on a [128,16]
# reshape of the 2048-sample row (full DVE/ACT lane use), then scattered into
# spec[4] partition 0 with an SBUF->SBUF reshape DMA; this removes the serial
# [1,W] tail that stalled the tensor engine in v3.
import numpy as np

import concourse.bacc as bacc
import concourse.mybir as mybir
import concourse.tile as tile
from concourse.bass_utils import run_bass_kernel_spmd

F32 = mybir.dt.float32
F16 = mybir.dt.float16
U16 = mybir.dt.uint16
ALU = mybir.AluOpType
ACTF = mybir.ActivationFunctionType

TWO_PI = 6.283185307179586
INV_2PI = 1.0 / TWO_PI
PI = 3.141592653589793
MAGIC16 = 1536.0  # 1.5 * 2**10: fp16 ulp 1 -> forces round-to-nearest-int
SIN_SCALE = 0.999999  # keeps rounding fuzz at +/-pi inside the Sin table domain
COS_BIAS = PI / 2 * SIN_SCALE

B_LOCAL = 2      # batches per core (16 total / 8 cores)
T = 2048         # STFT frames
SLICES = [(0, 256), (256, 256), (512, 512), (1024, 512), (1536, 512)]
N_CORES = 8


def _reg_const(nc, value, dtype=F32):
    key = (dtype, value)
    if key in nc.const_aps.aps:
        return
    t = nc.alloc_sbuf_tensor(f"const-{dtype.name}-{value}", [128, 1], dtype)
    nc.gpsimd.memset(t.ap(), value)
    nc.const_aps.aps[key] = t.ap()


def build_nc(repeat=1):
    nc = bacc.Bacc(target_bir_lowering=False)
    _reg_const(nc, COS_BIAS)
    nc.all_engine_barrier()
    mag = nc.declare_dram_parameter("mag", [B_LOCAL, 513, T], F16, isOutput=False)
    ang = nc.declare_dram_parameter("angle", [B_LOCAL, 513, T], F16, isOutput=False)
    invb = nc.declare_dram_parameter("invbasis", [1024, 1024], F16, isOutput=False)
    out = nc.declare_dram_parameter("out", [B_LOCAL, 523008], F16, isOutput=True)

    with tile.TileContext(nc) as tc:
        with (
            tc.tile_pool(name="const", bufs=1) as constp,
            tc.tile_pool(name="stage", bufs=4) as stagep,
            tc.tile_pool(name="spec", bufs=2) as specp,
            tc.tile_pool(name="work", bufs=3) as workp,
            tc.tile_pool(name="r5", bufs=2) as r5p,
            tc.tile_pool(name="osb", bufs=4) as osbp,
            tc.tile_pool(name="psum", bufs=8, space="PSUM") as psump,
        ):
            ib = [
                constp.tile([128, 1024], F16, tag=f"ib{q}", name=f"ib{q}")
                for q in range(8)
            ]
            for q in range(8):
                nc.scalar.dma_start(
                    out=ib[q][:, :], in_=invb[128 * q : 128 * (q + 1), :]
                )

            def ew_chain(at, mt, sv_out, cv_out, shp):
                """at/mt fp16 -> sv_out/cv_out fp16 (sin/cos times nothing yet).
                Returns (sv, cv) fp16 tiles of shape shp."""
                tb = workp.tile(shp, F16, tag="tb", name="tb")
                nc.vector.tensor_scalar(
                    out=tb, in0=at, scalar1=INV_2PI, scalar2=MAGIC16,
                    op0=ALU.mult, op1=ALU.add,
                )
                kk = workp.tile(shp, F16, tag="kk", name="kk")
                nc.vector.tensor_scalar_sub(kk, tb, MAGIC16)
                red = workp.tile(shp, F16, tag="red", name="red")
                nc.vector.scalar_tensor_tensor(
                    out=red, in0=kk, scalar=-TWO_PI, in1=at,
                    op0=ALU.mult, op1=ALU.add,
                )
                ared = workp.tile(shp, F16, tag="ared", name="ared")
                nc.vector.tensor_scalar(
                    out=ared.bitcast(U16), in0=red.bitcast(U16),
                    scalar1=0x7FFF, scalar2=None, op0=ALU.bitwise_and,
                )
                if sv_out is not None:
                    nc.scalar.activation(sv_out, red, ACTF.Sin, scale=SIN_SCALE)
                nc.scalar.activation(cv_out, ared, ACTF.Sin, scale=-SIN_SCALE,
                                     bias=COS_BIAS)

            def emit_group(b, spec, ct):
                """32 accumulating matmuls -> one [128,256] output tile -> DRAM."""
                ps = psump.tile([128, 256], F32, tag="ps", name="ps")
                c0 = 128 * ct
                mmi = 0
                for q in range(8):
                    for j in range(4):
                        nc.tensor.matmul(
                            out=ps[:, :],
                            lhsT=spec[q][:, c0 - j + 4 : c0 - j + 132],
                            rhs=ib[q][:, 256 * j : 256 * (j + 1)],
                            start=(mmi == 0),
                            stop=(mmi == 31),
                        )
                        mmi += 1
                ob = osbp.tile([128, 256], F16, tag="ob", name="ob")
                if ct % 2 == 0:
                    nc.scalar.activation(ob[:, :], ps[:, :], ACTF.Copy)
                else:
                    nc.vector.tensor_scalar_mul(ob[:, :], ps[:, :], 1.0)
                eng = nc.sync if ct % 2 == 0 else nc.scalar
                if ct == 0:
                    eng.dma_start(out=out[b, 0:31744], in_=ob[4:128, :])
                elif ct == 15:
                    lo = 256 * (128 * 15 - 4)
                    eng.dma_start(out=out[b, lo : lo + 32512], in_=ob[0:127, :])
                else:
                    lo = 256 * (128 * ct - 4)
                    eng.dma_start(out=out[b, lo : lo + 32768], in_=ob[:, :])

            for b in [b for _ in range(repeat) for b in range(B_LOCAL)]:
                # --- spec tiles: fp16 [128, 4 + T], 4 leading zero columns ---
                spec = []
                for q in range(8):
                    st = specp.tile([128, 4 + T], F16, tag=f"spec{q}", name=f"spec{q}")
                    nc.vector.memset(st[:, 0:4], 0.0)
                    spec.append(st)

                # --- row 512 (real only), full row as [128,16] ---
                m5 = r5p.tile([128, 16], F16, tag="m5", name="m5")
                a5 = r5p.tile([128, 16], F16, tag="a5", name="a5")
                nc.sync.dma_start(out=m5[:, :], in_=mag[b, 512, :])
                nc.sync.dma_start(out=a5[:, :], in_=ang[b, 512, :])
                cv5 = r5p.tile([128, 16], F16, tag="cv5", name="cv5")
                ew_chain(a5, m5, None, cv5, [128, 16])
                res5 = r5p.tile([128, 16], F16, tag="res5", name="res5")
                nc.vector.scalar_tensor_tensor(
                    out=res5, in0=m5, scalar=1.0, in1=cv5,
                    op0=ALU.mult, op1=ALU.mult,
                )

                emitted = 0
                for si, (s, W) in enumerate(SLICES):
                    cs = slice(4 + s, 4 + s + W)
                    for q in range(4):
                        rows = slice(128 * q, 128 * (q + 1))
                        mt = stagep.tile([128, W], F16, tag="mt", name="mt")
                        at = stagep.tile([128, W], F16, tag="at", name="at")
                        nc.sync.dma_start(out=mt[:, :], in_=mag[b, rows, s : s + W])
                        nc.sync.dma_start(out=at[:, :], in_=ang[b, rows, s : s + W])
                        sv = workp.tile([128, W], F16, tag="sv", name="sv")
                        cv = workp.tile([128, W], F16, tag="cv", name="cv")
                        ew_chain(at, mt, sv, cv, [128, W])
                        # real chunk q
                        nc.vector.scalar_tensor_tensor(
                            out=spec[q][:, cs], in0=mt, scalar=1.0, in1=cv,
                            op0=ALU.mult, op1=ALU.mult,
                        )
                        # imag chunk 4+q (for q=0 partition 0 gets garbage,
                        # overwritten below with the real row-512 values)
                        nc.vector.scalar_tensor_tensor(
                            out=spec[4 + q][:, cs], in0=mt, scalar=1.0, in1=sv,
                            op0=ALU.mult, op1=ALU.mult,
                        )
                    # patch spec[4] partition 0 for this slice from res5
                    # ([128,16] reshape of the row: sample 16p+i <-> (p, i))
                    nc.sync.dma_start(
                        out=spec[4][0:1, cs],
                        in_=res5[s // 16 : (s + W) // 16, :],
                    )

                    hi = min(16, (s + W - 132) // 128 + 1)
                    for ct in range(emitted, hi):
                        emit_group(b, spec, ct)
                    emitted = hi

                for ct in range(emitted, 16):
                    emit_group(b, spec, ct)
    nc.compile()
    return nc


_CACHE = {}


def _get_nc():
    if "nc" not in _CACHE:
        _CACHE["nc"] = build_nc()
    return _CACHE["nc"]


def make_in_maps(mag, angle, invbasis):
    """Host-side input marshalling: shard over cores, fp16-convert mag/angle,
    and pre-pack invbasis to the 1024-row fp16 layout (drops the two all-zero
    rows 513/1025; packed row 512+f <-> imag freq f, packed row 512 = real
    Nyquist row 512)."""
    mag = np.asarray(mag, dtype=np.float32).astype(np.float16)
    angle = np.asarray(angle, dtype=np.float32).astype(np.float16)
    invbasis = np.asarray(invbasis, dtype=np.float32)
    invb_packed = np.ascontiguousarray(
        np.concatenate([invbasis[:513], invbasis[514:1025]], axis=0).astype(np.float16)
    )
    return [
        {
            "mag": np.ascontiguousarray(mag[B_LOCAL * i : B_LOCAL * (i + 1)]),
            "angle": np.ascontiguousarray(angle[B_LOCAL * i : B_LOCAL * (i + 1)]),
            "invbasis": invb_packed,
        }
        for i in range(N_CORES)
    ]


def kernel(mag, angle, invbasis, _trace=False, **_ignored):
    nc = _get_nc()
    in_maps = make_in_maps(mag, angle, invbasis)
    res = run_bass_kernel_spmd(nc, in_maps, list(range(N_CORES)), trace=_trace)
    outs = [res.results[i]["out"] for i in range(N_CORES)]
    full = np.concatenate(outs, axis=0).astype(np.float32).reshape(16, 1, 523008)
    if _trace:
        return full, res
    return full
